# revision 1
# baseline (speedup 1.0000x reference)
"""Trainium2 kernel for nn_MultiHeadClassifier.

Math: out[i] = W[task_labels[i]] @ x[i] + b[task_labels[i]]
  x [262144, 1024] f32, task_labels [262144] int, W [8, 32, 1024], b [8, 32]

Strategy (8 NeuronCores, data-parallel over batch):
  - Each core gets 32768 rows. x is staged in HBM transposed
    ([8, 128, 32768]: k-tile, d-within-tile, row) so the PE can contract
    over d (partition dim) directly.
  - All T=8 heads are computed at once per 128-row tile: 8 float32r
    matmuls (full PE rate, ~1.5e-4 rel err) accumulate y = x @ Wflat.T
    ([128 rows, 256]) in PSUM, plus a K=1 bf16 matmul adding the bias.
  - Head selection (the MoE routing) happens on-device on the DVE:
    one-hot mask [128, 8] broadcast-multiplied into y viewed [128, 8, 32],
    then a strided reduce over the 8 task slots -> out tile [128, 32].
  - Output is written in [128, 256, 32] (partition-major) layout with
    fully contiguous per-partition DMA runs; host reshapes back.
"""

import sys

sys.path.insert(0, "/opt/trn_rl_repo")

import numpy as np
import ml_dtypes

import concourse.bass as bass
import concourse.tile as tile
from concourse import bacc, mybir
from concourse import bass_utils

B, D, C, T = 262144, 1024, 32, 8
NCORES = 8
N = B // NCORES  # 32768 rows per core
P = 128
KO = D // P  # 8 contraction tiles
TC = T * C  # 256 = all-heads output width
SB = 1024  # rows per superblock (one x DMA)
NT = N // P  # 256 row-tiles per core
SBT = SB // P  # row-tiles per superblock
NSB = N // SB  # superblocks per core

# set by test harness to collect a profile; harness-invoked kernel() keeps it off
TRACE = False
LAST_RESULTS = None


def _build():
    f32 = mybir.dt.float32
    f32r = mybir.dt.float32r
    bf16 = mybir.dt.bfloat16

    nc = bacc.Bacc("TRN2", debug=False, num_devices=NCORES)
    # xt[sb, ki, ko, r]: one superblock is a contiguous 2 MB region with
    # 16 KB contiguous per partition -> near-peak DMA efficiency.
    xt_d = nc.dram_tensor("xt", [NSB, P, KO, SB], f32r, kind="ExternalInput")
    wft_d = nc.dram_tensor("wft", [KO, P, TC], f32r, kind="ExternalInput")
    mask_d = nc.dram_tensor("mask8", [P, NT, T], f32, kind="ExternalInput")
    # bpack[0, :P] = ones, bpack[0, P:] = b.reshape(256) twice (bf16)
    bpack_d = nc.dram_tensor("bpack", [1, P + 2 * TC], bf16, kind="ExternalInput")
    out_d = nc.dram_tensor("out", [P, NT, C], f32, kind="ExternalOutput")

    with tile.TileContext(nc) as tc:
        with (
            tc.tile_pool(name="consts", bufs=1) as consts,
            tc.tile_pool(name="xpool", bufs=5) as xpool,
            tc.tile_pool(name="work", bufs=8) as work,
            tc.tile_pool(name="opool", bufs=3) as opool,
            tc.tile_pool(name="psum", bufs=8, space="PSUM") as psum,
        ):
            # first x superblock in flight before the consts
            xts0 = xpool.tile([P, KO, SB], f32r, tag="xts")
            nc.sync.dma_start(xts0[:], xt_d[0])

            # consts on the ACT ring: the SP ring stays a pure x stream
            wft = consts.tile([P, KO, TC], f32r)
            nc.scalar.dma_start(wft[:], wft_d[:].rearrange("ko ki n -> ki ko n"))
            mask8 = consts.tile([P, NT, T], f32)
            nc.scalar.dma_start(mask8[:], mask_d[:])
            bpack = consts.tile([1, P + 2 * TC], bf16)
            nc.scalar.dma_start(bpack[:], bpack_d[:])
            ones1 = bpack[:, :P]
            bexp2 = bpack[:, P:]  # [1, 512] = b flat, twice

            # Engine warmups: with the 1-sync-wait-per-instruction ISA
            # limit, give each engine one instruction that observes the
            # const DMA lanes, so steady-state instructions carry at most
            # one wait each.
            scratch = psum.tile([P, TC], mybir.dt.float32, tag="y")
            w0 = wft[:, 0, :1].bitcast(bf16)  # [P, 2] garbage bf16 view
            nc.tensor.matmul(scratch[:2, :2], w0, w0, start=True, stop=True)
            dve_scr = work.tile([P, T], f32, tag="dve_scr")
            nc.vector.tensor_copy(dve_scr[:], mask8[:, 0, :])

            for sb in range(NSB):
                if sb == 0:
                    xts = xts0
                else:
                    xts = xpool.tile([P, KO, SB], f32r, tag="xts")
                    nc.sync.dma_start(xts[:], xt_d[sb])
                out_sb = opool.tile([P, SBT, C], f32, tag="out_sb")
                for st in range(SBT):
                    ro = sb * SBT + st
                    y = psum.tile([P, TC], mybir.dt.float32, tag="y")
                    # bias first: absorbs the psum-slot WAR wait; single
                    # const producer (bpack DMA).
                    nc.tensor.matmul(
                        y[:], ones1, bexp2[:, :TC], start=True, stop=False
                    )
                    for ko in range(KO):
                        nc.tensor.matmul(
                            y[:],
                            xts[:, ko, st * P : (st + 1) * P],
                            wft[:, ko, :],
                            start=False,
                            stop=(ko == KO - 1),
                        )
                    # tmp[p, t, c] = y[p, t*C+c] * mask8[p, ro, t]
                    tmp = work.tile([P, TC], f32, tag="tmp")
                    nc.vector.tensor_tensor(
                        tmp[:].rearrange("p (t c) -> p t c", t=T),
                        y[:].rearrange("p (t c) -> p t c", t=T),
                        mask8[:, ro, :, None].to_broadcast((P, T, C)),
                        mybir.AluOpType.mult,
                    )
                    # out[p, c] = sum_t tmp[p, t, c]
                    nc.vector.tensor_reduce(
                        out_sb[:, st, :],
                        tmp[:].rearrange("p (t c) -> p c t", t=T),
                        axis=mybir.AxisListType.X,
                        op=mybir.AluOpType.add,
                    )
                # out on the ACT HWDGE ring so it never delays xts loads
                # queued on the SP ring
                nc.scalar.dma_start(
                    out_d[:, sb * SBT : (sb + 1) * SBT, :], out_sb[:]
                )
    nc.compile()
    return nc


_NC = None


def _get_nc():
    global _NC
    if _NC is None:
        _NC = _build()
    return _NC


def kernel(x, task_labels, W, b):
    global LAST_RESULTS
    x = np.asarray(x)
    if x.dtype != np.float32:
        x = x.astype(np.float32)
    labels = np.asarray(task_labels).astype(np.int32)
    W = np.asarray(W)
    if W.dtype != np.float32:
        W = W.astype(np.float32)
    b = np.asarray(b)
    if b.dtype != np.float32:
        b = b.astype(np.float32)

    wft = np.ascontiguousarray(W.reshape(TC, D).T).reshape(KO, P, TC)
    bpack = (
        np.concatenate(
            [np.ones(P, np.float32), b.reshape(TC), b.reshape(TC)]
        )
        .reshape(1, P + 2 * TC)
        .astype(ml_dtypes.bfloat16)
    )
    tids = np.arange(T, dtype=np.int32)[None, None, :]

    in_maps = []
    for c in range(NCORES):
        xs = x[c * N : (c + 1) * N]
        ls = labels[c * N : (c + 1) * N]
        # xt[sb, ki, ko, r] = xs[sb*SB + r, ko*P + ki]
        xt = np.ascontiguousarray(
            xs.reshape(NSB, SB, KO, P).transpose(0, 3, 2, 1)
        )
        lab2 = ls.reshape(NT, P).T  # [P, NT]
        mask8 = (lab2[:, :, None] == tids).astype(np.float32)
        in_maps.append(
            {"xt": xt, "wft": wft, "mask8": mask8, "bpack": bpack}
        )

    nc = _get_nc()
    res = bass_utils.run_bass_kernel_spmd(
        nc, in_maps, core_ids=list(range(NCORES)), trace=TRACE
    )
    LAST_RESULTS = res
    outs = [
        r["out"].transpose(1, 0, 2).reshape(N, C) for r in res.results
    ]
    return np.concatenate(outs, axis=0)



# revision 3
# speedup vs baseline: 1.9835x; 1.9835x over previous
"""Trainium2 kernel for nn_MultiHeadClassifier.

Math: out[i] = W[task_labels[i]] @ x[i] + b[task_labels[i]]
  x [262144, 1024] f32, task_labels [262144] int, W [8, 32, 1024], b [8, 32]

Strategy (8 NeuronCores): shard by TASK, not by batch position. Core c
receives exactly the rows with task_labels == c (counts are ~32768 +- 200,
so the load is balanced), permuted host-side. Every core then runs a
single dense GEMM against its own head's weights -- no routing, no mask,
no per-row dispatch on device at all. The host un-permutes the result and
adds the bias during the scatter.

The problem is HBM-bound (x alone is 1 GiB). x and W are cast to fp16 on
the host, halving device HBM traffic vs f32 (rel err ~6e-4, fine for the
2e-2 gate). Per core: ~68 MB x in + ~4 MB out => ~190 us roofline at
358 GB/s per-core HBM bandwidth.

Device kernel per core:
  - x staged transposed [ki=128, ko=8, R rows] fp16 so the PE can contract
    over d directly; streamed in 2048-row superblocks (4 KB contiguous
    runs per (partition, ko)).
  - W_head.T staged [ki, ko, 32] fp16; the stationary operand is only 32
    columns wide => LDWEIGHTS is ~27 ns, and each matmul streams 512 rows
    (moving operand) into a [32, 512] f32 PSUM tile, accumulating over the
    8 ko tiles. PE time ~110 us < DMA time => stays DMA-bound.
  - DVE drains PSUM -> SBUF [32, 2048] f32, one out DMA per superblock on
    the scalar ring ([32, R] f32, 8 KB runs); host transposes back.
"""

import sys

sys.path.insert(0, "/opt/trn_rl_repo")

import numpy as np

import concourse.bass as bass
import concourse.tile as tile
from concourse import bacc, mybir
from concourse import bass_utils

B, D, C, T = 262144, 1024, 32, 8
NCORES = 8
P = 128
KO = D // P  # 8 contraction tiles
SB = 2048  # rows per superblock (one x DMA)
BLK = 512  # rows per matmul / PSUM tile

# set by test harness to collect a profile; harness-invoked kernel() keeps it off
TRACE = False
LAST_RESULTS = None

_XDT = mybir.dt.float16
_XNP = np.float16


def _build(R):
    """Compile the SPMD program for R padded rows per core."""
    f32 = mybir.dt.float32

    superblocks = [SB] * (R // SB)
    if R % SB:
        superblocks.append(R % SB)

    nc = bacc.Bacc("TRN2", debug=False, num_devices=NCORES)
    xt_d = nc.dram_tensor("xt", [P, KO, R], _XDT, kind="ExternalInput")
    wt_d = nc.dram_tensor("wt", [P, KO, C], _XDT, kind="ExternalInput")
    out_d = nc.dram_tensor("out", [C, R], f32, kind="ExternalOutput")

    with tile.TileContext(nc) as tc:
        with (
            tc.tile_pool(name="consts", bufs=1) as consts,
            tc.tile_pool(name="xpool", bufs=4) as xpool,
            tc.tile_pool(name="opool", bufs=3) as opool,
            tc.tile_pool(name="psum", bufs=8, space="PSUM") as psum,
        ):
            # first x superblock in flight before the consts
            xts0 = xpool.tile([P, KO, SB], _XDT, tag="xts")
            nc.sync.dma_start(xts0[:, :, : superblocks[0]], xt_d[:, :, : superblocks[0]])

            # consts on the ACT ring: the SP ring stays a pure x stream
            wt = consts.tile([P, KO, C], _XDT)
            nc.scalar.dma_start(wt[:], wt_d[:])

            # PE warmup observing the wt DMA lane, so steady-state matmuls
            # carry at most one sync wait each.
            scratch = psum.tile([C, BLK], f32, tag="y")
            nc.tensor.matmul(
                scratch[:2, :2], wt[:, 0, :2], wt[:, 0, :2], start=True, stop=True
            )

            r0 = 0
            for sb, rows in enumerate(superblocks):
                if sb == 0:
                    xts = xts0
                else:
                    xts = xpool.tile([P, KO, SB], _XDT, tag="xts")
                    nc.sync.dma_start(xts[:, :, :rows], xt_d[:, :, r0 : r0 + rows])
                out_sb = opool.tile([C, SB], f32, tag="out_sb")
                for blk in range(rows // BLK):
                    y = psum.tile([C, BLK], f32, tag="y")
                    for ko in range(KO):
                        nc.tensor.matmul(
                            y[:],
                            wt[:, ko, :],
                            xts[:, ko, blk * BLK : (blk + 1) * BLK],
                            start=(ko == 0),
                            stop=(ko == KO - 1),
                        )
                    nc.vector.tensor_copy(
                        out_sb[:, blk * BLK : (blk + 1) * BLK], y[:]
                    )
                # out on the ACT HWDGE ring so it never delays xts loads
                nc.scalar.dma_start(out_d[:, r0 : r0 + rows], out_sb[:, :rows])
                r0 += rows
    nc.compile()
    return nc


_NC_CACHE = {}


def _get_nc(R):
    if R not in _NC_CACHE:
        _NC_CACHE[R] = _build(R)
    return _NC_CACHE[R]


def _R_for(labels):
    counts = np.bincount(np.asarray(labels).astype(np.int64), minlength=T)
    return -(-max(int(counts.max()), 1) // BLK) * BLK  # pad to a BLK multiple


def kernel(x, task_labels, W, b):
    global LAST_RESULTS
    x = np.asarray(x)
    if x.dtype != np.float32:
        x = x.astype(np.float32)
    labels = np.asarray(task_labels).astype(np.int64)
    W = np.asarray(W).astype(np.float32)
    b = np.asarray(b).astype(np.float32)

    # route rows to cores by task
    idxs = [np.nonzero(labels == c)[0] for c in range(T)]
    counts = [len(ix) for ix in idxs]
    R = _R_for(labels)

    in_maps = []
    for c in range(NCORES):
        xp = np.zeros((R, D), dtype=_XNP)
        xp[: counts[c]] = x[idxs[c]]
        # xt[ki, ko, r] = xp[r, ko*P + ki]
        xt = np.ascontiguousarray(xp.reshape(R, KO, P).transpose(2, 1, 0))
        # wt[ki, ko, cc] = W[c][cc, ko*P + ki]
        wt = np.ascontiguousarray(
            W[c].T.reshape(KO, P, C).transpose(1, 0, 2)
        ).astype(_XNP)
        in_maps.append({"xt": xt, "wt": wt})

    nc = _get_nc(R)
    res = bass_utils.run_bass_kernel_spmd(
        nc, in_maps, core_ids=list(range(NCORES)), trace=TRACE
    )
    LAST_RESULTS = res

    out = np.empty((B, C), dtype=np.float32)
    for c in range(NCORES):
        oc = res.results[c]["out"]  # [C, R] f32
        out[idxs[c]] = oc[:, : counts[c]].T + b[c]
    return out


# revision 7
# speedup vs baseline: 2.0961x; 1.0567x over previous
"""Trainium2 kernel for nn_MultiHeadClassifier.

Math: out[i] = W[task_labels[i]] @ x[i] + b[task_labels[i]]
  x [262144, 1024] f32, task_labels [262144] int, W [8, 32, 1024], b [8, 32]

Strategy (8 NeuronCores): shard by TASK, not by batch position. Core c
receives exactly the rows with task_labels == c (counts are ~32768 +- 200,
so the load is balanced), permuted host-side. Every core then runs a
single dense GEMM against its own head's weights -- no routing, no mask,
no per-row dispatch on device at all. The host un-permutes the result and
adds the bias during the scatter.

The problem is HBM-bound (x alone is 1 GiB). x and W are cast to fp16 on
the host, halving device HBM traffic vs f32 (rel err ~6e-4, fine for the
2e-2 gate). Per core: ~68 MB x in + ~4 MB out => ~190 us roofline at
358 GB/s per-core HBM bandwidth.

Device kernel per core:
  - x staged transposed [ki=128, ko=8, R rows] fp16 so the PE can contract
    over d directly; streamed in 2048-row superblocks (4 KB contiguous
    runs per (partition, ko)).
  - W_head.T staged [ki, ko, 32] fp16; the stationary operand is only 32
    columns wide => LDWEIGHTS is ~27 ns, and each matmul streams 512 rows
    (moving operand) into a [32, 512] f32 PSUM tile, accumulating over the
    8 ko tiles. PE time ~110 us < DMA time => stays DMA-bound.
  - DVE drains PSUM -> SBUF [32, 2048] f32, one out DMA per superblock on
    the scalar ring ([32, R] f32, 8 KB runs); host transposes back.
"""

import sys

sys.path.insert(0, "/opt/trn_rl_repo")

import numpy as np

import concourse.bass as bass
import concourse.tile as tile
from concourse import bacc, mybir
from concourse import bass_utils

B, D, C, T = 262144, 1024, 32, 8
NCORES = 8
P = 128
KO = D // P  # 8 contraction tiles
SB = 2048  # rows per superblock (one x DMA)
BLK = 512  # rows per matmul / PSUM tile

# set by test harness to collect a profile; harness-invoked kernel() keeps it off
TRACE = False
LAST_RESULTS = None

_XDT = mybir.dt.float16
_XNP = np.float16


def _superblocks(R):
    """Big x DMAs in steady state, tapered small ones at the end so the
    serial compute tail after the last DMA is short."""
    sbs = []
    rem = R
    while rem > 3 * SB // 2:
        sbs.append(SB)
        rem -= SB
    while rem >= BLK:
        sbs.append(BLK)
        rem -= BLK
    if rem:
        sbs.append(rem)
    return sbs


def _build(R):
    """Compile the SPMD program for R padded rows per core."""
    f32 = mybir.dt.float32

    superblocks = _superblocks(R)

    nc = bacc.Bacc("TRN2", debug=False, num_devices=NCORES)
    xt_d = nc.dram_tensor("xt", [P, KO, R], _XDT, kind="ExternalInput")
    wt_d = nc.dram_tensor("wt", [P, KO, C], _XDT, kind="ExternalInput")
    out_d = nc.dram_tensor("out", [C, R], _XDT, kind="ExternalOutput")

    with tile.TileContext(nc) as tc:
        with (
            tc.tile_pool(name="consts", bufs=1) as consts,
            tc.tile_pool(name="xpool", bufs=4) as xpool,
            tc.tile_pool(name="opool", bufs=3) as opool,
            tc.tile_pool(name="psum", bufs=8, space="PSUM") as psum,
        ):
            # first x superblock in flight before the consts
            xts0 = xpool.tile([P, KO, SB], _XDT, tag="xts")
            nc.sync.dma_start(xts0[:, :, : superblocks[0]], xt_d[:, :, : superblocks[0]])

            # consts on the ACT ring: the SP ring stays a pure x stream
            wt = consts.tile([P, KO, C], _XDT)
            nc.scalar.dma_start(wt[:], wt_d[:])

            # PE warmup observing the wt DMA lane, so steady-state matmuls
            # carry at most one sync wait each.
            scratch = psum.tile([C, BLK], f32, tag="y")
            nc.tensor.matmul(
                scratch[:2, :2], wt[:, 0, :2], wt[:, 0, :2], start=True, stop=True
            )

            r0 = 0
            for sb, rows in enumerate(superblocks):
                if sb == 0:
                    xts = xts0
                else:
                    xts = xpool.tile([P, KO, SB], _XDT, tag="xts")
                    nc.sync.dma_start(xts[:, :, :rows], xt_d[:, :, r0 : r0 + rows])
                out_sb = opool.tile([C, SB], _XDT, tag="out_sb")
                for b0 in range(0, rows, BLK):
                    n = min(BLK, rows - b0)
                    y = psum.tile([C, BLK], f32, tag="y")
                    for ko in range(KO):
                        nc.tensor.matmul(
                            y[:, :n],
                            wt[:, ko, :],
                            xts[:, ko, b0 : b0 + n],
                            start=(ko == 0),
                            stop=(ko == KO - 1),
                        )
                    nc.vector.tensor_copy(out_sb[:, b0 : b0 + n], y[:, :n])
                # out on the ACT HWDGE ring so it never delays xts loads
                nc.scalar.dma_start(out_d[:, r0 : r0 + rows], out_sb[:, :rows])
                r0 += rows
    nc.compile()
    return nc


_NC_CACHE = {}


def _get_nc(R):
    if R not in _NC_CACHE:
        _NC_CACHE[R] = _build(R)
    return _NC_CACHE[R]


def _R_for(labels):
    counts = np.bincount(np.asarray(labels).astype(np.int64), minlength=T)
    return -(-max(int(counts.max()), 1) // P) * P  # pad to a partition multiple


def kernel(x, task_labels, W, b):
    global LAST_RESULTS
    x = np.asarray(x)
    if x.dtype != np.float32:
        x = x.astype(np.float32)
    labels = np.asarray(task_labels).astype(np.int64)
    W = np.asarray(W).astype(np.float32)
    b = np.asarray(b).astype(np.float32)

    # route rows to cores by task
    idxs = [np.nonzero(labels == c)[0] for c in range(T)]
    counts = [len(ix) for ix in idxs]
    R = _R_for(labels)

    in_maps = []
    for c in range(NCORES):
        xp = np.zeros((R, D), dtype=_XNP)
        xp[: counts[c]] = x[idxs[c]]
        # xt[ki, ko, r] = xp[r, ko*P + ki]
        xt = np.ascontiguousarray(xp.reshape(R, KO, P).transpose(2, 1, 0))
        # wt[ki, ko, cc] = W[c][cc, ko*P + ki]
        wt = np.ascontiguousarray(
            W[c].T.reshape(KO, P, C).transpose(1, 0, 2)
        ).astype(_XNP)
        in_maps.append({"xt": xt, "wt": wt})

    nc = _get_nc(R)
    res = bass_utils.run_bass_kernel_spmd(
        nc, in_maps, core_ids=list(range(NCORES)), trace=TRACE
    )
    LAST_RESULTS = res

    out = np.empty((B, C), dtype=np.float32)
    for c in range(NCORES):
        oc = res.results[c]["out"]  # [C, R] fp16
        out[idxs[c]] = oc[:, : counts[c]].T.astype(np.float32) + b[c]
    return out
